# revision 1
# baseline (speedup 1.0000x reference)
"""Trainium2 Bass kernel for sparse incremental attention (nn_Attention_4415226380332).

Computes, for b=16 batches sharded 2-per-core across 8 NeuronCores:
    A = softmax(K^T Q / sqrt(d), axis=N)         [b, N, T]
    (incremental) A[:, :, :ct] = previous_att[:, :, :ct]
    argmax/force correction of column ct          (done on host, O(b*N))
    R = V @ A                                     [b, d, T]

Device kernel (per batch):
  - S[n,t] = sum_d K[d,n] Q[d,t] for t >= ct only (prefix columns are never
    softmaxed - they are overwritten by previous_att).
  - E = exp(S * scale) on ScalarE (no max subtraction needed: |S*scale| < ~6).
  - column sums via ones-vector matmul on TensorE, reciprocal on VectorE,
    replicated across partitions with a K=1 matmul, normalize on VectorE.
  - V transposed on TensorE (32x [128,128] tiles) to give lhsT for R.
  - R = Vt.T @ [prefix | A] accumulated over 8 n-chunks per output tile.
"""

import math
import os

import numpy as np

_CACHE = {}
LAST_RESULTS = None  # BassKernelResults of the most recent device run (for test.py)

_N_CORES = 8
_P = 128
_FREE = 512  # matmul moving-operand chunk (fp32 max, one PSUM bank)


def _softmax_np(x, axis):
    m = x.max(axis=axis, keepdims=True)
    e = np.exp(x - m)
    return e / e.sum(axis=axis, keepdims=True)


def _reference_numpy(K, V, Q, force_incremental, previous_position, previous_att,
                     current_time):
    """Pure-numpy fallback, mirrors reference.py exactly. Used only for input
    configurations the device kernel doesn't support."""
    b, d, N = K.shape
    scale = 1.0 / np.sqrt(d).astype(np.float32)
    A = _softmax_np((np.einsum('bdn,bdt->bnt', K, Q) * scale).astype(np.float32), 1)
    current_position = A[:, :, current_time].argmax(1)
    if force_incremental:
        A[:, :, :current_time] = previous_att[:, :, :current_time]
        difference = current_position - previous_position
        force_needed = (difference < -1) | (difference > 3)
        idx = np.clip(previous_position + 1, 0, N - 1)
        col = A[:, :, current_time].copy()
        for bb in range(b):
            if force_needed[bb]:
                col[bb] = 0.0
                col[bb, idx[bb]] = 1.0
        A[:, :, current_time] = col
        current_position = col.argmax(1)
    R = np.einsum('bdn,bnt->bdt', V, A).astype(np.float32)
    return A, R, current_position.astype(np.int32)


def _build(bpc, d, N, T, t0):
    """Build + compile the per-core Bass module.

    bpc: batches per core. t0: first computed column (multiple of 512; columns
    [0, t0) are copied from previous_att). Computed region is [t0, T).
    """
    import concourse.bacc as bacc
    import concourse.bass as bass
    import concourse.mybir as mybir
    import concourse.tile as tile
    from concourse.masks import make_identity

    f32 = mybir.dt.float32
    DC = d // _P            # d chunks (partition dim of K/Q/V)
    NCH = N // _P           # n chunks
    Th = T - t0             # computed width
    TC = Th // _FREE        # computed t-chunks
    ATC = T // _FREE        # all t-chunks (for R)
    PC = t0 // _FREE        # prefix t-chunks
    scale = 1.0 / math.sqrt(d)

    nc = bacc.Bacc("TRN2", target_bir_lowering=False, debug=False,
                   num_devices=_N_CORES)

    K_d = nc.dram_tensor("K", [bpc, d, N], f32, kind="ExternalInput").ap()
    Q_d = nc.dram_tensor("Q", [bpc, d, Th], f32, kind="ExternalInput").ap()
    V_d = nc.dram_tensor("V", [bpc, d, N], f32, kind="ExternalInput").ap()
    P_d = None
    if t0 > 0:
        P_d = nc.dram_tensor("PATT", [bpc, N, t0], f32, kind="ExternalInput").ap()
    A_d = nc.dram_tensor("A", [bpc, N, T], f32, kind="ExternalOutput").ap()
    R_d = nc.dram_tensor("R", [bpc, d, T], f32, kind="ExternalOutput").ap()

    with tile.TileContext(nc) as tc:
        with (
            tc.tile_pool(name="const", bufs=1) as constp,
            tc.tile_pool(name="kq", bufs=1) as kqp,
            tc.tile_pool(name="ea", bufs=1) as eap,
            tc.tile_pool(name="vv", bufs=1) as vvp,
            tc.tile_pool(name="pp", bufs=1) as ppp,
            tc.tile_pool(name="rr", bufs=3) as rrp,
            tc.tile_pool(name="sm", bufs=2) as smp,
            tc.tile_pool(name="ps_s", bufs=2, space=bass.MemorySpace.PSUM) as ps_s,
            tc.tile_pool(name="ps_cs", bufs=1, space=bass.MemorySpace.PSUM) as ps_cs,
            tc.tile_pool(name="ps_rb", bufs=1, space=bass.MemorySpace.PSUM) as ps_rb,
            tc.tile_pool(name="ps_tp", bufs=2, space=bass.MemorySpace.PSUM) as ps_tp,
            tc.tile_pool(name="ps_r", bufs=2, space=bass.MemorySpace.PSUM) as ps_r,
        ):
            identity = constp.tile([_P, _P], f32)
            make_identity(nc, identity)
            ones_col = constp.tile([_P, 1], f32)
            nc.vector.memset(ones_col, 1.0)
            ones_row = constp.tile([1, _P], f32)
            nc.vector.memset(ones_row, 1.0)

            for bi in range(bpc):
                # ---- load K, Q (tiled [128, chunk, free]) ----
                K_t = kqp.tile([_P, DC, N], f32, tag="K")
                nc.sync.dma_start(K_t[:], K_d[bi].rearrange("(c p) n -> p c n", p=_P))
                Q_t = kqp.tile([_P, DC, Th], f32, tag="Q")
                nc.sync.dma_start(Q_t[:], Q_d[bi].rearrange("(c p) t -> p c t", p=_P))

                # ---- S = K^T Q, E = exp(S*scale), colsums, normalize ----
                EA = eap.tile([_P, NCH, Th], f32, tag="EA")
                rrow = smp.tile([1, Th], f32, tag="rrow")
                rb = smp.tile([_P, Th], f32, tag="rb")
                for tci in range(TC):
                    tsl = slice(tci * _FREE, (tci + 1) * _FREE)
                    cs_ps = ps_cs.tile([1, _FREE], f32, tag="cs")
                    for ni in range(NCH):
                        s_ps = ps_s.tile([_P, _FREE], f32, tag="s")
                        for ki in range(DC):
                            nc.tensor.matmul(
                                s_ps[:],
                                K_t[:, ki, ni * _P:(ni + 1) * _P],
                                Q_t[:, ki, tsl],
                                start=(ki == 0), stop=(ki == DC - 1),
                            )
                        nc.scalar.activation(
                            EA[:, ni, tsl], s_ps[:],
                            mybir.ActivationFunctionType.Exp, scale=scale,
                        )
                        nc.tensor.matmul(
                            cs_ps[:], ones_col[:], EA[:, ni, tsl],
                            start=(ni == 0), stop=(ni == NCH - 1),
                        )
                    nc.vector.reciprocal(rrow[:, tsl], cs_ps[:])
                    rb_ps = ps_rb.tile([_P, _FREE], f32, tag="rb")
                    nc.tensor.matmul(rb_ps[:], ones_row[:], rrow[:, tsl])
                    nc.vector.tensor_copy(rb[:, tsl], rb_ps[:])
                    for ni in range(NCH):
                        nc.vector.tensor_mul(EA[:, ni, tsl], EA[:, ni, tsl],
                                             rb[:, tsl])

                # ---- write computed half of A ----
                nc.sync.dma_start(
                    A_d[bi].rearrange("(c p) t -> p c t", p=_P)[:, :, t0:T], EA[:])

                # ---- V load + transpose to [n, d] ----
                V_t = vvp.tile([_P, DC, N], f32, tag="V")
                nc.sync.dma_start(V_t[:], V_d[bi].rearrange("(c p) n -> p c n", p=_P))
                Vt = vvp.tile([_P, NCH, d], f32, tag="Vt")
                for ni in range(NCH):
                    for ci in range(DC):
                        tp_ps = ps_tp.tile([_P, _P], f32, tag="tp")
                        nc.tensor.transpose(
                            tp_ps[:], V_t[:, ci, ni * _P:(ni + 1) * _P], identity[:])
                        nc.vector.tensor_copy(
                            Vt[:, ni, ci * _P:(ci + 1) * _P], tp_ps[:])

                # ---- prefix: load previous_att[:, :t0], copy straight to A ----
                P_t = None
                if t0 > 0:
                    P_t = ppp.tile([_P, NCH, t0], f32, tag="P")
                    nc.sync.dma_start(
                        P_t[:], P_d[bi].rearrange("(c p) t -> p c t", p=_P))
                    nc.sync.dma_start(
                        A_d[bi].rearrange("(c p) t -> p c t", p=_P)[:, :, 0:t0],
                        P_t[:])

                # ---- R = Vt.T @ [prefix | A] ----
                for ci in range(DC):
                    R_st = rrp.tile([_P, T], f32, tag="R")
                    for tj in range(ATC):
                        r_ps = ps_r.tile([_P, _FREE], f32, tag="r")
                        for ni in range(NCH):
                            if tj < PC:
                                rhs = P_t[:, ni, tj * _FREE:(tj + 1) * _FREE]
                            else:
                                rhs = EA[:, ni,
                                         tj * _FREE - t0:(tj + 1) * _FREE - t0]
                            nc.tensor.matmul(
                                r_ps[:], Vt[:, ni, ci * _P:(ci + 1) * _P], rhs,
                                start=(ni == 0), stop=(ni == NCH - 1),
                            )
                        nc.scalar.copy(R_st[:, tj * _FREE:(tj + 1) * _FREE], r_ps[:])
                    nc.sync.dma_start(R_d[bi, ci * _P:(ci + 1) * _P, :], R_st[:])

    nc.compile()
    return nc


def _get_compiled(bpc, d, N, T, t0):
    key = (bpc, d, N, T, t0)
    if key not in _CACHE:
        _CACHE[key] = _build(*key)
    return _CACHE[key]


def kernel(K, V, Q, force_incremental, previous_position, previous_att,
           current_time):
    global LAST_RESULTS
    K = np.ascontiguousarray(np.asarray(K, dtype=np.float32))
    V = np.ascontiguousarray(np.asarray(V, dtype=np.float32))
    Q = np.ascontiguousarray(np.asarray(Q, dtype=np.float32))
    previous_att = np.asarray(previous_att, dtype=np.float32)
    prev_pos = np.asarray(previous_position)
    inc = bool(int(force_incremental))
    ct = int(current_time)

    b, d, N = K.shape
    T = Q.shape[2]
    t0 = ct if inc else 0

    unsupported = (
        d % _P != 0 or N % _P != 0 or T % _FREE != 0 or t0 % _FREE != 0
        or not (0 <= t0 <= T) or (inc and not 0 <= ct < T)
    )
    if unsupported:
        return _reference_numpy(K, V, Q, inc, prev_pos, previous_att, ct)

    # pad batch count to a multiple of the core count
    bpc = (b + _N_CORES - 1) // _N_CORES
    b_pad = bpc * _N_CORES
    if b_pad != b:
        pad = ((0, b_pad - b),) + ((0, 0),) * 2
        K = np.pad(K, pad)
        V = np.pad(V, pad)
        Q = np.pad(Q, pad)
        if t0 > 0:
            previous_att = np.pad(previous_att, pad)

    nc = _get_compiled(bpc, d, N, T, t0)

    Qh = Q[:, :, t0:]
    in_maps = []
    for c in range(_N_CORES):
        sl = slice(c * bpc, (c + 1) * bpc)
        m = {"K": K[sl], "Q": np.ascontiguousarray(Qh[sl]), "V": V[sl]}
        if t0 > 0:
            m["PATT"] = np.ascontiguousarray(previous_att[sl, :, :t0])
        in_maps.append(m)

    from concourse.bass_utils import run_bass_kernel_spmd
    res = run_bass_kernel_spmd(nc, in_maps, core_ids=list(range(_N_CORES)))
    LAST_RESULTS = res

    A = np.concatenate([r["A"] for r in res.results], axis=0)[:b]
    R = np.concatenate([r["R"] for r in res.results], axis=0)[:b]

    # host-side argmax + force correction of column ct (O(b*N) work)
    col_raw = A[:, :, ct]
    cp_raw = col_raw.argmax(axis=1).astype(np.int64)
    if inc:
        diff = cp_raw - prev_pos.astype(np.int64)
        force = (diff < -1) | (diff > 3)
        idx = np.clip(prev_pos.astype(np.int64) + 1, 0, N - 1)
        for bb in range(b):
            if force[bb]:
                A[bb, :, ct] = 0.0
                A[bb, idx[bb], ct] = 1.0
                R[bb, :, ct] = V[bb, :, idx[bb]]
        cp = np.where(force, idx, cp_raw).astype(np.int32)
    else:
        cp = cp_raw.astype(np.int32)
    return A, R, cp


# revision 3
# speedup vs baseline: 8.3342x; 8.3342x over previous
"""Trainium2 Bass kernel for sparse incremental attention (nn_Attention_4415226380332).

Computes, for b=16 batches sharded 2-per-core across 8 NeuronCores:
    A = softmax(K^T Q / sqrt(d), axis=N)         [b, N, T]
    (incremental) A[:, :, :ct] = previous_att[:, :, :ct]
    argmax/force correction of column ct          (done on host, O(b*N))
    R = V @ A                                     [b, d, T]

Device kernel (per batch):
  - S[n,t] = sum_d K[d,n] Q[d,t] for t >= ct only (prefix columns are never
    softmaxed - they are overwritten by previous_att).
  - All matmul operands are float32r (TF32-like, ~1.2e-4 rounding): fp32
    matmul on the PE costs 4 cycles/row, fp32r costs 1 - this kernel is
    PE-bound at fp32 (~390us/core) vs DMA-bound at fp32r (~125us/core).
  - E = exp(S * scale) on ScalarE (no max subtraction needed: |S*scale| < ~6).
  - column sums via ones-vector matmul on TensorE, reciprocal on VectorE,
    replicated across partitions with a K=1 matmul, normalize on VectorE.
  - V transposed on TensorE (32x [128,128] tiles) to give lhsT for R.
  - R = Vt.T @ [prefix | A] accumulated over 8 n-chunks per output tile.
"""

import math

import numpy as np

_CACHE = {}
LAST_RESULTS = None  # BassKernelResults of the most recent device run (for test.py)

_N_CORES = 8
_P = 128
_FREE = 512  # matmul moving-operand chunk (one PSUM bank of fp32)


def _softmax_np(x, axis):
    m = x.max(axis=axis, keepdims=True)
    e = np.exp(x - m)
    return e / e.sum(axis=axis, keepdims=True)


def _reference_numpy(K, V, Q, force_incremental, previous_position, previous_att,
                     current_time):
    """Pure-numpy fallback, mirrors reference.py exactly. Used only for input
    configurations the device kernel doesn't support."""
    b, d, N = K.shape
    scale = np.float32(1.0 / np.sqrt(d))
    A = _softmax_np((np.einsum('bdn,bdt->bnt', K, Q) * scale).astype(np.float32), 1)
    current_position = A[:, :, current_time].argmax(1)
    if force_incremental:
        A[:, :, :current_time] = previous_att[:, :, :current_time]
        difference = current_position - previous_position
        force_needed = (difference < -1) | (difference > 3)
        idx = np.clip(previous_position + 1, 0, N - 1)
        col = A[:, :, current_time].copy()
        for bb in range(b):
            if force_needed[bb]:
                col[bb] = 0.0
                col[bb, idx[bb]] = 1.0
        A[:, :, current_time] = col
        current_position = col.argmax(1)
    R = np.einsum('bdn,bnt->bdt', V, A).astype(np.float32)
    return A, R, current_position.astype(np.int32)


def _build(bpc, d, N, T, t0, repeat=1):
    """Build + compile the per-core Bass module.

    bpc: batches per core. t0: first computed column (multiple of 512; columns
    [0, t0) are copied from previous_att). repeat: wrap the whole body in a
    hardware loop (benchmarking only - outputs are just rewritten each time).
    """
    import concourse.bacc as bacc
    import concourse.bass as bass
    import concourse.mybir as mybir
    import concourse.tile as tile
    from concourse.masks import make_identity

    f32 = mybir.dt.float32
    f32r = mybir.dt.float32r
    DC = d // _P            # d chunks (partition dim of K/Q/V)
    NCH = N // _P           # n chunks
    Th = T - t0             # computed width
    TC = Th // _FREE        # computed t-chunks
    ATC = T // _FREE        # all t-chunks (for R)
    PC = t0 // _FREE        # prefix t-chunks
    scale = 1.0 / math.sqrt(d)

    nc = bacc.Bacc("TRN2", target_bir_lowering=False, debug=False,
                   num_devices=_N_CORES)

    K_d = nc.dram_tensor("K", [bpc, d, N], f32, kind="ExternalInput").ap()
    Q_d = nc.dram_tensor("Q", [bpc, d, Th], f32, kind="ExternalInput").ap()
    V_d = nc.dram_tensor("V", [bpc, d, N], f32, kind="ExternalInput").ap()
    P_d = None
    if t0 > 0:
        P_d = nc.dram_tensor("PATT", [bpc, N, t0], f32, kind="ExternalInput").ap()
    A_d = nc.dram_tensor("A", [bpc, N, T], f32, kind="ExternalOutput").ap()
    R_d = nc.dram_tensor("R", [bpc, d, T], f32, kind="ExternalOutput").ap()

    with tile.TileContext(nc) as tc:
        with (
            tc.tile_pool(name="const", bufs=1) as constp,
            tc.tile_pool(name="kq", bufs=1) as kqp,
            tc.tile_pool(name="ea", bufs=1) as eap,
            tc.tile_pool(name="vv", bufs=1) as vvp,
            tc.tile_pool(name="pp", bufs=1) as ppp,
            tc.tile_pool(name="rr", bufs=3) as rrp,
            tc.tile_pool(name="sm", bufs=2) as smp,
            tc.tile_pool(name="ps_s", bufs=2, space=bass.MemorySpace.PSUM) as ps_s,
            tc.tile_pool(name="ps_cs", bufs=1, space=bass.MemorySpace.PSUM) as ps_cs,
            tc.tile_pool(name="ps_rb", bufs=1, space=bass.MemorySpace.PSUM) as ps_rb,
            tc.tile_pool(name="ps_tp", bufs=2, space=bass.MemorySpace.PSUM) as ps_tp,
            tc.tile_pool(name="ps_r", bufs=2, space=bass.MemorySpace.PSUM) as ps_r,
        ):
            ident_f = constp.tile([_P, _P], f32)
            make_identity(nc, ident_f)
            identity = constp.tile([_P, _P], f32r)
            nc.vector.tensor_copy(identity[:], ident_f[:])
            ones_col_f = constp.tile([_P, 1], f32)
            nc.vector.memset(ones_col_f, 1.0)
            ones_col = constp.tile([_P, 1], f32r)
            nc.vector.tensor_copy(ones_col[:], ones_col_f[:])
            ones_row_f = constp.tile([1, _P], f32)
            nc.vector.memset(ones_row_f, 1.0)
            ones_row = constp.tile([1, _P], f32r)
            nc.vector.tensor_copy(ones_row[:], ones_row_f[:])

            def body(_iv=None):
                for bi in range(bpc):
                    # ---- load K, Q (tiled [128, chunk, free], f32->f32r) ----
                    K_t = kqp.tile([_P, DC, N], f32r, tag="K")
                    nc.gpsimd.dma_start(
                        K_t[:], K_d[bi].rearrange("(c p) n -> p c n", p=_P))
                    Q_t = kqp.tile([_P, DC, Th], f32r, tag="Q")
                    nc.gpsimd.dma_start(
                        Q_t[:], Q_d[bi].rearrange("(c p) t -> p c t", p=_P))

                    # ---- S = K^T Q, E = exp(S*scale), colsums, normalize ----
                    EA = eap.tile([_P, NCH, Th], f32r, tag="EA")
                    rrow = smp.tile([1, Th], f32r, tag="rrow")
                    rb = smp.tile([_P, Th], f32r, tag="rb")
                    for tci in range(TC):
                        tsl = slice(tci * _FREE, (tci + 1) * _FREE)
                        cs_ps = ps_cs.tile([1, _FREE], f32, tag="cs")
                        for ni in range(NCH):
                            s_ps = ps_s.tile([_P, _FREE], f32, tag="s")
                            for ki in range(DC):
                                nc.tensor.matmul(
                                    s_ps[:],
                                    K_t[:, ki, ni * _P:(ni + 1) * _P],
                                    Q_t[:, ki, tsl],
                                    start=(ki == 0), stop=(ki == DC - 1),
                                )
                            nc.scalar.activation(
                                EA[:, ni, tsl], s_ps[:],
                                mybir.ActivationFunctionType.Exp, scale=scale,
                            )
                            nc.tensor.matmul(
                                cs_ps[:], ones_col[:], EA[:, ni, tsl],
                                start=(ni == 0), stop=(ni == NCH - 1),
                            )
                        with nc.allow_low_precision(reason="f32r softmax scale"):
                            nc.vector.reciprocal(rrow[:, tsl], cs_ps[:])
                        rb_ps = ps_rb.tile([_P, _FREE], f32, tag="rb")
                        nc.tensor.matmul(rb_ps[:], ones_row[:], rrow[:, tsl])
                        nc.vector.tensor_copy(rb[:, tsl], rb_ps[:])
                        for ni in range(NCH):
                            nc.vector.tensor_mul(EA[:, ni, tsl], EA[:, ni, tsl],
                                                 rb[:, tsl])

                    # ---- write computed half of A ----
                    nc.sync.dma_start(
                        A_d[bi].rearrange("(c p) t -> p c t", p=_P)[:, :, t0:T],
                        EA[:].bitcast(f32))

                    # ---- V load + transpose to [n, d] ----
                    V_t = vvp.tile([_P, DC, N], f32r, tag="V")
                    nc.gpsimd.dma_start(
                        V_t[:], V_d[bi].rearrange("(c p) n -> p c n", p=_P))
                    Vt = vvp.tile([_P, NCH, d], f32r, tag="Vt")
                    for ni in range(NCH):
                        for ci in range(DC):
                            tp_ps = ps_tp.tile([_P, _P], f32r, tag="tp")
                            nc.tensor.transpose(
                                tp_ps[:], V_t[:, ci, ni * _P:(ni + 1) * _P],
                                identity[:])
                            nc.vector.tensor_copy(
                                Vt[:, ni, ci * _P:(ci + 1) * _P], tp_ps[:])

                    # ---- prefix: load previous_att[:, :t0], copy out to A ----
                    P_t = None
                    if t0 > 0:
                        P_t = ppp.tile([_P, NCH, t0], f32r, tag="P")
                        nc.gpsimd.dma_start(
                            P_t[:], P_d[bi].rearrange("(c p) t -> p c t", p=_P))
                        nc.sync.dma_start(
                            A_d[bi].rearrange("(c p) t -> p c t", p=_P)[:, :, 0:t0],
                            P_t[:].bitcast(f32))

                    # ---- R = Vt.T @ [prefix | A] ----
                    for ci in range(DC):
                        R_st = rrp.tile([_P, T], f32, tag="R")
                        for tj in range(ATC):
                            r_ps = ps_r.tile([_P, _FREE], f32, tag="r")
                            for ni in range(NCH):
                                if tj < PC:
                                    rhs = P_t[:, ni, tj * _FREE:(tj + 1) * _FREE]
                                else:
                                    rhs = EA[:, ni,
                                             tj * _FREE - t0:(tj + 1) * _FREE - t0]
                                nc.tensor.matmul(
                                    r_ps[:], Vt[:, ni, ci * _P:(ci + 1) * _P], rhs,
                                    start=(ni == 0), stop=(ni == NCH - 1),
                                )
                            nc.scalar.copy(
                                R_st[:, tj * _FREE:(tj + 1) * _FREE], r_ps[:])
                        nc.sync.dma_start(R_d[bi, ci * _P:(ci + 1) * _P, :], R_st[:])

            if repeat == 1:
                body()
            else:
                with tc.For_i(0, repeat, 1) as _i:
                    body(_i)

    nc.compile()
    return nc


def _get_compiled(bpc, d, N, T, t0):
    key = (bpc, d, N, T, t0)
    if key not in _CACHE:
        _CACHE[key] = _build(*key)
    return _CACHE[key]


def kernel(K, V, Q, force_incremental, previous_position, previous_att,
           current_time):
    global LAST_RESULTS
    K = np.ascontiguousarray(np.asarray(K, dtype=np.float32))
    V = np.ascontiguousarray(np.asarray(V, dtype=np.float32))
    Q = np.ascontiguousarray(np.asarray(Q, dtype=np.float32))
    previous_att = np.asarray(previous_att, dtype=np.float32)
    prev_pos = np.asarray(previous_position)
    inc = bool(int(force_incremental))
    ct = int(current_time)

    b, d, N = K.shape
    T = Q.shape[2]
    t0 = ct if inc else 0

    unsupported = (
        d % _P != 0 or N % _P != 0 or T % _FREE != 0 or t0 % _FREE != 0
        or not (0 <= t0 <= T) or (inc and not 0 <= ct < T)
    )
    if unsupported:
        return _reference_numpy(K, V, Q, inc, prev_pos, previous_att, ct)

    # pad batch count to a multiple of the core count
    bpc = (b + _N_CORES - 1) // _N_CORES
    b_pad = bpc * _N_CORES
    if b_pad != b:
        pad = ((0, b_pad - b),) + ((0, 0),) * 2
        K = np.pad(K, pad)
        V = np.pad(V, pad)
        Q = np.pad(Q, pad)
        if t0 > 0:
            previous_att = np.pad(previous_att, pad)

    nc = _get_compiled(bpc, d, N, T, t0)

    Qh = Q[:, :, t0:]
    in_maps = []
    for c in range(_N_CORES):
        sl = slice(c * bpc, (c + 1) * bpc)
        m = {"K": K[sl], "Q": np.ascontiguousarray(Qh[sl]), "V": V[sl]}
        if t0 > 0:
            m["PATT"] = np.ascontiguousarray(previous_att[sl, :, :t0])
        in_maps.append(m)

    from concourse.bass_utils import run_bass_kernel_spmd
    res = run_bass_kernel_spmd(nc, in_maps, core_ids=list(range(_N_CORES)))
    LAST_RESULTS = res

    A = np.concatenate([r["A"] for r in res.results], axis=0)[:b]
    R = np.concatenate([r["R"] for r in res.results], axis=0)[:b]

    # host-side argmax + force correction of column ct (O(b*N) work)
    col_raw = A[:, :, ct]
    cp_raw = col_raw.argmax(axis=1).astype(np.int64)
    if inc:
        diff = cp_raw - prev_pos.astype(np.int64)
        force = (diff < -1) | (diff > 3)
        idx = np.clip(prev_pos.astype(np.int64) + 1, 0, N - 1)
        for bb in range(b):
            if force[bb]:
                A[bb, :, ct] = 0.0
                A[bb, idx[bb], ct] = 1.0
                R[bb, :, ct] = V[bb, :, idx[bb]]
        cp = np.where(force, idx, cp_raw).astype(np.int32)
    else:
        cp = cp_raw.astype(np.int32)
    return A, R, cp
